# revision 7
# baseline (speedup 1.0000x reference)
"""Multi-head linear attention (Performer/FAVOR+) Bass kernel for 8x TRN2 cores.

Sharding: 8 cores = 4 batches x 2 head-groups. Core c handles batch c//2 and
heads [4*(c%2), 4*(c%2)+4).

Math notes (exact rewrites of the reference, not approximations):
  - omega is sqrt(64) * orthogonal, so 0.5*||q||^2 = ||q @ Omega.T||^2 / 128:
    the squared-sum term is computed from xw itself.
  - The per-row scale exp(-qsq_t), the global 1/sqrt(128) scale and
    (approximately) the +EPS term all cancel in out = qkv[..,:64]/qkv[..,64],
    so the q-side feature map is just exp(+-xw).
  - The k-side scale rho_s = exp(-ksq_s) is folded into v1 = [v, 1]*rho so
    kp is also just exp(+-kxw).

v2 structure (heads processed in PAIRS of two):
  - Q proj packs a head pair into one 128-col stationary [wqo_he | wqo_ho]:
    one N=512 matmul per (pair, chunk, tchunk) - half the PE work of the
    [w,-w] trick. The +- split becomes two ACT exps (same element count).
  - kv accumulation is TRANSPOSED: stationary = v1 (65 cols -> cheap
    LDWEIGHTS), moving = kp (N=128): kvT[d(65), h, sign, m(64)] accumulates
    in one PSUM bank over all 32 s-tiles. At the end, 8 tiny matmuls against
    an identity build block-diagonal KVP/KVN [128m, 130d] per pair
    (rows 0:64 = even head's features, 64:128 = odd head's; bank-clear on the
    first matmul zeroes the off-diagonal blocks).
  - qkv per (t-tile, pair): qp_pos.T @ KVP + qp_neg.T @ KVN -> [128t, 130]
    = both heads' 65 cols (64 values + normalizer).
  - Normalization happens ON HOST: the kernel DMAs bf16 rows
    [t, (pair, head, 65)] and the host divides values by the normalizer.
  - ksq: Square on DVE (tensor_mul) + reduce_sum on DVE, keeping ACT (the
    near-critical engine) free for the exps.
  - Input DMA: few large transfers in consumption order (k/v/q segments
    interleaved) instead of ~100 small ones.
"""

import sys

import numpy as np

for _p in ("/opt/trn_rl_repo", "/root/.axon_site/_ro/trn_rl_repo"):
    try:
        import concourse  # noqa: F401
        break
    except ImportError:
        if _p not in sys.path:
            sys.path.insert(0, _p)

B, T, D, H = 4, 4096, 512, 8
DK = DV = 64
HPC = 4            # heads per core
NPAIR = 2          # head pairs per core
NCH = 4            # f chunks (512 / 128)
P = 128
ST = T // P        # 32 s-tiles
TC = 8             # t chunks
TCW = T // TC      # 512
DVN = DV + 1       # 65: values + normalizer

_CACHE = {}


def _build_program(reps=1):
    import concourse.mybir as mybir
    import concourse.tile as tile
    from concourse import bacc
    from contextlib import ExitStack

    dt = mybir.dt

    nc = bacc.Bacc("TRN2", target_bir_lowering=False, debug=False)

    qt_d = nc.dram_tensor("qt", [D, T], dt.float16, kind="ExternalInput")
    kt_d = nc.dram_tensor("kt", [D, T], dt.float16, kind="ExternalInput")
    vt_d = nc.dram_tensor("vt", [D, T], dt.float16, kind="ExternalInput")
    wqp_d = nc.dram_tensor("wqp", [NPAIR, NCH, P, P], dt.float16,
                           kind="ExternalInput")
    wko_d = nc.dram_tensor("wko", [NCH, P, HPC * DK], dt.float16,
                           kind="ExternalInput")
    wv_d = nc.dram_tensor("wv", [NCH, P, HPC * DV], dt.float16,
                          kind="ExternalInput")
    id_d = nc.dram_tensor("ident", [DVN, DVN], dt.bfloat16,
                          kind="ExternalInput")
    out_d = nc.dram_tensor("out", [T * HPC * DVN], dt.bfloat16,
                           kind="ExternalOutput")

    with tile.TileContext(nc) as tc, ExitStack() as ctx:
        const = ctx.enter_context(tc.tile_pool(name="const", bufs=1))
        work = ctx.enter_context(tc.tile_pool(name="work", bufs=3))
        psum = ctx.enter_context(tc.tile_pool(name="psum", bufs=1, space="PSUM"))
        for _rep in range(reps):
            _emit_body(nc, tc, const, work, psum, mybir, dt,
                       qt_d, kt_d, vt_d, wqp_d, wko_d, wv_d, id_d, out_d)

    nc.compile()
    return nc


def _emit_body(nc, tc, const, work, psum, mybir, dt,
               qt_d, kt_d, vt_d, wqp_d, wko_d, wv_d, id_d, out_d):
    AF = mybir.ActivationFunctionType

    # persistent SBUF residents
    qt = const.tile([P, NCH, T], dt.float16)
    kt = const.tile([P, NCH, T], dt.float16)
    vt = const.tile([P, NCH, T], dt.float16)
    wqp = const.tile([P, NPAIR, NCH, P], dt.float16)
    wko = const.tile([P, NCH, HPC * DK], dt.float16)
    wv = const.tile([P, NCH, HPC * DV], dt.float16)
    ident = const.tile([DVN, DVN], dt.bfloat16)
    kvt_sb = const.tile([DVN, HPC, 2, DK], dt.bfloat16)
    kvp_sb = const.tile([P, NPAIR, 2 * DVN], dt.bfloat16)
    kvn_sb = const.tile([P, NPAIR, 2 * DVN], dt.bfloat16)

    # Input DMA: big transfers issued in the order the loop consumes them.
    nc.sync.dma_start(out=wko[:], in_=wko_d.ap().rearrange("c p n -> p c n"))
    nc.sync.dma_start(out=wv[:], in_=wv_d.ap().rearrange("c p n -> p c n"))
    nc.sync.dma_start(out=ident[:], in_=id_d[:, :])
    kt_r = kt_d.ap().rearrange("(c p) t -> p c t", c=NCH)
    vt_r = vt_d.ap().rearrange("(c p) t -> p c t", c=NCH)
    qt_r = qt_d.ap().rearrange("(c p) t -> p c t", c=NCH)
    segs = [(0, 512), (512, 1536), (1536, 2560), (2560, 3584), (3584, T)]
    s0, e0 = segs[0]
    nc.sync.dma_start(out=kt[:, :, s0:e0], in_=kt_r[:, :, s0:e0])
    nc.sync.dma_start(out=vt[:, :, s0:e0], in_=vt_r[:, :, s0:e0])
    nc.sync.dma_start(out=wqp[:], in_=wqp_d.ap().rearrange("r c p m -> p r c m"))
    nc.sync.dma_start(out=qt[:, :, s0:e0], in_=qt_r[:, :, s0:e0])
    for s, e in segs[1:]:
        nc.sync.dma_start(out=kt[:, :, s:e], in_=kt_r[:, :, s:e])
        nc.sync.dma_start(out=vt[:, :, s:e], in_=vt_r[:, :, s:e])
        nc.sync.dma_start(out=qt[:, :, s:e], in_=qt_r[:, :, s:e])

    # kvT accumulator: one PSUM bank, [d(65), h, sign, m(64)] fp32.
    # All matmuls use start=False and accumulate; memset clears the bank
    # (overwrite-on-clear / add-on-set per-element semantics).
    kvt_ps = psum.tile([DVN, HPC, 2, DK], dt.float32, tag="kvtacc", bufs=1)
    nc.vector.memset(kvt_ps[:], 0.0)

    qp_tiles = []   # per tchunk: (qp_pos[2], qp_neg[2])

    def emit_q_chunk(tcx):
        tsl = slice(tcx * TCW, (tcx + 1) * TCW)
        ptiles, ntiles = [], []
        for pr in range(NPAIR):
            qx = psum.tile([P, TCW], dt.float32, tag="qx", bufs=2, name="qx")
            for c in range(NCH):
                nc.tensor.matmul(
                    qx[:], wqp[:, pr, c, :], qt[:, c, tsl],
                    start=(c == 0), stop=(c == NCH - 1),
                )
            qpp = work.tile([P, TCW], dt.bfloat16, tag="qpp", bufs=2 * TC,
                            name=f"qpp{tcx}_{pr}")
            qpn = work.tile([P, TCW], dt.bfloat16, tag="qpn", bufs=2 * TC,
                            name=f"qpn{tcx}_{pr}")
            nc.scalar.activation(qpp[:], qx[:], AF.Exp)
            nc.scalar.activation(qpn[:], qx[:], AF.Exp, scale=-1.0)
            ptiles.append(qpp)
            ntiles.append(qpn)
        qp_tiles.append((ptiles, ntiles))

    # ---------------- phase KV (with q-projection work interleaved) -----
    # Two s-tiles per "pair" iteration. Engines execute queues IN ORDER;
    # cross-engine deps are software-pipelined: v1 (needs rho from ACT) is
    # emitted one iteration late on DVE, the kvT matmuls (need v1) one
    # iteration later still on PE.
    NP_ = ST // 2    # 16 iterations
    stage = {}

    def emit_v1(pi):
        st_ = stage[pi]
        v1 = work.tile([P, 2, HPC, DVN], dt.bfloat16, tag="v1", name="v1")
        nc.vector.tensor_mul(
            v1[:, :, :, 0:DV], st_["v_ps"],
            st_["rho"][:].broadcast_to([P, 2, HPC, DV])
        )
        nc.vector.tensor_copy(v1[:, :, :, DV:DVN], st_["rho"][:])
        st_["v1"] = v1

    def emit_kvt(pi):
        st_ = stage.pop(pi)
        for p_ in range(2):
            si = 2 * pi + p_
            for h in range(HPC):
                # out = v1_h.T @ kp_h : [65, (sign,64)]
                nc.tensor.matmul(
                    kvt_ps[:, h, :, :], st_["v1"][:, p_, h, :],
                    st_["kp"][:, p_, :, h * DK:(h + 1) * DK],
                    start=False, stop=(si == ST - 1),
                    skip_group_check=True,
                )

    for pi in range(NP_):
        # kxw and v share PSUM banks: [..., 0:64] = kxw, 64:128 = v
        kxwv = psum.tile([P, 2, HPC, 2 * DK], dt.float32, tag="kxwv",
                         bufs=2, name="kxwv")
        kxw = kxwv[:, :, :, 0:DK]
        v_ps = kxwv[:, :, :, DK:2 * DK]
        for p_ in range(2):
            ssl = slice((2 * pi + p_) * P, (2 * pi + p_ + 1) * P)
            for c in range(NCH):
                nc.tensor.matmul(
                    kxwv[:, p_, :, 0:DK], kt[:, c, ssl], wko[:, c, :],
                    start=(c == 0), stop=(c == NCH - 1),
                )
            for c in range(NCH):
                nc.tensor.matmul(
                    kxwv[:, p_, :, DK:2 * DK], vt[:, c, ssl], wv[:, c, :],
                    start=(c == 0), stop=(c == NCH - 1),
                )
        if pi >= 2:
            emit_kvt(pi - 2)

        # kp: [s, p_, sign, m(4h x 64)]
        kp = work.tile([P, 2, 2, HPC * DK], dt.bfloat16, tag="kp", bufs=3)
        nc.scalar.activation(kp[:, :, 0, :], kxw, AF.Exp, scale=1.0)
        nc.scalar.activation(kp[:, :, 1, :], kxw, AF.Exp, scale=-1.0)

        # ksq: ACT Square into PSUM (ScE->PSUM is its fast path; DVE cannot
        # read two PSUM operands so a DVE square is not possible), then DVE
        # reduce from PSUM.
        sqsc = psum.tile([P, 2, HPC, DK], dt.float32, tag="sq", bufs=1,
                         name="sqsc")
        nc.scalar.activation(sqsc[:], kxw, AF.Square)
        ksqr = work.tile([P, 2, HPC, 1], dt.float32, tag="ksqr")
        nc.vector.reduce_sum(ksqr[:], sqsc[:], axis=mybir.AxisListType.X)
        rho = work.tile([P, 2, HPC, 1], dt.float32, tag="rho")
        nc.scalar.activation(rho[:], ksqr[:], AF.Exp, scale=-1.0 / 128.0)

        stage[pi] = {"v_ps": v_ps, "rho": rho, "kp": kp}
        if pi >= 1:
            emit_v1(pi - 1)

        if pi % 2 == 1:
            emit_q_chunk(pi // 2)

    emit_v1(NP_ - 1)
    emit_kvt(NP_ - 2)
    emit_kvt(NP_ - 1)

    # ---------------- kv fixup: kvT -> block-diagonal KVP / KVN ---------
    # memset zeroes the data; matmuls accumulate with start=False (correct
    # for both stale has_written states); the never-written off-diagonal
    # blocks keep the memset zeros.
    nc.vector.tensor_copy(kvt_sb[:], kvt_ps[:])
    for pr in range(NPAIR):
        he, ho = 2 * pr, 2 * pr + 1
        kvx_ps = psum.tile([P, 2 * DVN], dt.float32, tag="qx", bufs=2,
                           name=f"kvx{pr}")
        nc.vector.memset(kvx_ps[:], 0.0)
        nc.tensor.matmul(kvx_ps[0:DK, 0:DVN],
                         kvt_sb[:, he, 0, :], ident[:],
                         start=False, stop=False, skip_group_check=True)
        nc.tensor.matmul(kvx_ps[DK:P, DVN:2 * DVN],
                         kvt_sb[:, ho, 0, :], ident[:],
                         start=False, stop=True,
                         skip_group_check=True, tile_position=(0, DK))
        nc.vector.tensor_copy(kvp_sb[:, pr, :], kvx_ps[:])
        kvy_ps = psum.tile([P, 2 * DVN], dt.float32, tag="qx", bufs=2,
                           name=f"kvy{pr}")
        nc.vector.memset(kvy_ps[:], 0.0)
        nc.tensor.matmul(kvy_ps[0:DK, 0:DVN],
                         kvt_sb[:, he, 1, :], ident[:],
                         start=False, stop=False, skip_group_check=True)
        nc.tensor.matmul(kvy_ps[DK:P, DVN:2 * DVN],
                         kvt_sb[:, ho, 1, :], ident[:],
                         start=False, stop=True,
                         skip_group_check=True, tile_position=(0, DK))
        nc.vector.tensor_copy(kvn_sb[:, pr, :], kvy_ps[:])

    # ---------------- tail: qkv + store (normalization on host) ---------
    # qkv PSUM tile: [t, tti, pair, 256-pad] = 2 banks; each (tti, pair)
    # block is 130 fp32 inside its own 1 KiB half-bank, so no matmul output
    # straddles a bank. start=True on each bank's first matmul clears that
    # bank's has_written; the other blocks overwrite-on-clear / add-on-set.
    for tcx in range(TC):
        ptiles, ntiles = qp_tiles[tcx]
        oq = work.tile([P, 4, NPAIR, 2 * DVN], dt.bfloat16, tag="oq", bufs=3)
        for th in range(2):      # two tt-halves of 2 t-tiles each
            qkv = psum.tile([P, 2, NPAIR, 2 * P], dt.float32,
                            tag="kxwv", bufs=2, name="qkv")
            for tti in range(2):
                tt = 2 * th + tti
                ttsl = slice(tt * P, (tt + 1) * P)
                for pr in range(NPAIR):
                    nc.tensor.matmul(
                        qkv[:, tti, pr, 0:2 * DVN], ptiles[pr][:, ttsl],
                        kvp_sb[:, pr, :],
                        start=(pr == 0), stop=False,
                        skip_group_check=True,
                    )
                    nc.tensor.matmul(
                        qkv[:, tti, pr, 0:2 * DVN], ntiles[pr][:, ttsl],
                        kvn_sb[:, pr, :],
                        start=False, stop=(pr == NPAIR - 1),
                        skip_group_check=True,
                    )
            nc.vector.tensor_copy(oq[:, 2 * th:2 * th + 2, :, :],
                                  qkv[:, :, :, 0:2 * DVN])
        ofs = tcx * TCW * HPC * DVN
        nc.sync.dma_start(
            out=out_d.ap()[ofs:ofs + TCW * HPC * DVN].rearrange(
                "(tt p d) -> p tt d", tt=4, p=P
            ),
            in_=oq[:],
        )


def _get_program(reps=1):
    if reps not in _CACHE:
        _CACHE[reps] = _build_program(reps)
    return _CACHE[reps]


def _prep_core_inputs(query, value, key, wqo, wko, wv_w, core):
    b, hg = core // 2, core % 2
    hs = slice(hg * HPC, (hg + 1) * HPC)

    qT = np.ascontiguousarray(query[b].T.astype(np.float16))   # (512, 4096)
    kT = np.ascontiguousarray(key[b].T.astype(np.float16))
    vT = np.ascontiguousarray(value[b].T.astype(np.float16))

    wqo_c = wqo[hs]                                            # (4, 512, 64)
    wqp = np.stack([
        np.concatenate([wqo_c[2 * pr], wqo_c[2 * pr + 1]], axis=1)
        for pr in range(NPAIR)
    ])                                                         # (2, 512, 128)
    wqp = np.ascontiguousarray(
        wqp.reshape(NPAIR, NCH, P, P).astype(np.float16))      # (pr, c, p, m)

    wko_c = np.concatenate(list(wko[hs]), axis=1)              # (512, 256)
    wko_c = np.ascontiguousarray(
        wko_c.reshape(NCH, P, HPC * DK).astype(np.float16))
    wv_c = np.concatenate(list(wv_w[hs]), axis=1)              # (512, 256)
    wv_c = np.ascontiguousarray(
        wv_c.reshape(NCH, P, HPC * DV).astype(np.float16))

    import ml_dtypes
    ident = np.eye(DVN, dtype=ml_dtypes.bfloat16)

    return {"qt": qT, "kt": kT, "vt": vT,
            "wqp": wqp, "wko": wko_c, "wv": wv_c, "ident": ident}


def kernel(query, value, key, wq, wv, wk, omega):
    from concourse.bass_utils import run_bass_kernel_spmd

    query = np.asarray(query, np.float32)
    value = np.asarray(value, np.float32)
    key = np.asarray(key, np.float32)
    wq = np.asarray(wq, np.float32)
    wv = np.asarray(wv, np.float32)
    wk = np.asarray(wk, np.float32)
    omega = np.asarray(omega, np.float32)

    nc = _get_program()

    wqo = np.einsum("hfk,mk->hfm", wq, omega)                  # (8, 512, 64)
    wko = np.einsum("hfk,mk->hfm", wk, omega)

    in_maps = [
        _prep_core_inputs(query, value, key, wqo, wko, wv, core)
        for core in range(8)
    ]
    res = run_bass_kernel_spmd(nc, in_maps, core_ids=list(range(8)))

    out = np.empty((B, T, D), np.float32)
    for core in range(8):
        b, hg = core // 2, core % 2
        qkv = np.asarray(res.results[core]["out"], np.float32)
        qkv = qkv.reshape(T, HPC, DVN).transpose(1, 0, 2)      # (h, t, 65)
        vals = qkv[:, :, :DV] / qkv[:, :, DV:DVN]              # (4, 4096, 64)
        out[b, hg * 2048:(hg + 1) * 2048, :] = vals.reshape(2048, D)
    return out
